# revision 49
# baseline (speedup 1.0000x reference)
"""DCT-II enhancement kernel for Trainium2 (8 NeuronCores, data parallel).

Computes out[b, n, k] = sum_d x[b, n, d] * C[k, d] where C is the 256x256
orthonormal DCT-II basis — i.e. a [B*N, 256] @ [256, 256]^T GEMM.

Sharding: pure data parallel over the flattened token dim (B*N = 131072),
16384 tokens per core.

Precision (harness gate: rel_err < 2e-2):
  x:   bf16  (input quantization ~1e-3)
  ct:  bf16  (basis must stay >= bf16; an fp8 basis hits e3m4 subnormals)
  acc: fp32 PSUM
  out: fp8 e3m4 (total rel err 1.363e-2, measured on the deterministic
       harness data — jax.random.key(0))
HBM traffic per core: 8.39 MB in + 4.19 MB out = 12.6 MB (vs 33.6 fp32).
(Sending a token fraction as fp8 with a split hi+lo e3m4 basis was tried
and is numerically fine at 1.52e-2, but the 2x PE streaming for those
tokens makes the kernel PE-bound and net ~8us slower — rejected.)

Layout trick: the host pre-transposes each shard to xT[d, tok] (d on
partitions) and post-transposes the result, so the device does NO
transposes — just matmuls with the tiny DCT basis stationary:

  outT[kb*128+kp, t] = sum_c ct_chunk[c, :, kb].T @ xT_chunk[c, :, t]

Per-core dataflow, per slab (graduated sizes 256..2048..512: small at
the head so the first matmul starts ~10us in, small at the tail to cut
the post-last-input drain chain):
  A(i): DMA xT tile [128p(d), 2c, S] from HBM (up to 4 KB runs per
        partition), alternating sync(HWDGE)/gpsimd(SWDGE) queues so the
        SDMA engines interleave two read streams with the write stream.
  B(i): per <=512-token tile x 2 k-blocks: 2 accumulating bf16 matmuls
        into a PSUM bank [128(k), <=512(t)]; PSUM fp32 -> SBUF fp8 cast
        copies (alternating DVE/ACT); out-DMA per slab on the scalar
        ring, per-tile on the (by then idle) sync ring for the last 3
        slabs.

Measured (core 0 NTFF, 8 cores concurrent): min 49.0us, median ~53us
over 8 samples, vs 104.5us for the fp32 baseline. Steady-state DMA is
99%+ packed at ~341 GB/s; the remaining fixed costs are ~3us of ramp
and ~8.5us of tail (final DMA receipt + the codegen-emitted
253-semaphore clear postamble).
"""

from contextlib import ExitStack

import ml_dtypes
import numpy as np

import concourse.bass as bass
import concourse.tile as tile
from concourse import bacc, mybir
from concourse.bass_utils import run_bass_kernel_spmd

P = 128
D = 256
N_CORES = 8
B, N = 32, 4096
TOK = (B * N) // N_CORES  # 16384 tokens per core
C = D // P                # 2 contraction chunks of 128
KB = D // P               # 2 output k-blocks of 128

TILE = 512                # tokens per PSUM tile (one bank: 512 fp32)
# Graduated slabs: small at the head (fast pipeline fill -> first matmul
# sooner) and at the tail (short post-last-input drain chain).
SLABS = [256, 256, 512, 1024] + [2048] * 6 + [1024, 512, 512]  # sum 16384
OFF = [sum(SLABS[:i]) for i in range(len(SLABS))]
NSLAB = len(SLABS)

BF16 = mybir.dt.bfloat16
F32 = mybir.dt.float32
FP8 = mybir.dt.float8e3

BF16_NP = ml_dtypes.bfloat16
FP8_NP = ml_dtypes.float8_e3m4  # wire format of the "out" tensor


def dct_matrix() -> np.ndarray:
    """C[k, d] — DCT-II with ortho normalization, fp64 math cast to fp32."""
    n = D
    k = np.arange(n)[:, None].astype(np.float64)
    m = np.arange(n)[None, :].astype(np.float64)
    Cm = np.cos(np.pi * (2.0 * m + 1.0) * k / (2.0 * n))
    scale = np.full((n, 1), np.sqrt(2.0 / n))
    scale[0, 0] = np.sqrt(1.0 / n)
    return (Cm * scale).astype(np.float32)


def build_program(num_devices: int = N_CORES) -> bass.Bass:
    """Emit the per-core Bass/Tile program. All cores run the same NEFF."""
    nc = bacc.Bacc(
        "TRN2", target_bir_lowering=False, debug=False, num_devices=num_devices
    )
    xt_d = nc.dram_tensor("xt", [C, P, TOK], BF16, kind="ExternalInput").ap()
    # ct packed [p, c, k] host-side: one contiguous 1 KB run per partition.
    ct_d = nc.dram_tensor("ct", [P, C, D], BF16, kind="ExternalInput").ap()
    out_d = nc.dram_tensor("out", [KB, P, TOK], FP8, kind="ExternalOutput").ap()

    with ExitStack() as ctx:
        tc = ctx.enter_context(tile.TileContext(nc))
        consts = ctx.enter_context(tc.tile_pool(name="consts", bufs=1))
        xin_pool = ctx.enter_context(tc.tile_pool(name="xin", bufs=4))
        out_sb_pool = ctx.enter_context(tc.tile_pool(name="out_sb", bufs=4))
        out_ps_pool = ctx.enter_context(
            tc.tile_pool(name="out_ps", bufs=8, space="PSUM")
        )

        # Basis on the scalar ring so the sync ring starts streaming x
        # immediately; flat layout -> fast descriptors -> first MM sooner.
        ct_sb = consts.tile([P, C, D], BF16)
        nc.scalar.dma_start(ct_sb[:], ct_d)

        xt_r = xt_d.rearrange("c p t -> p c t")    # [128, 2, TOK]
        o_r = out_d.rearrange("c p t -> p c t")    # [128, 2, TOK]

        xins: dict = {}

        def stage_in(i):
            if not (0 <= i < NSLAB):
                return
            t0, s = OFF[i], SLABS[i]
            xin = xin_pool.tile([P, C, s], BF16)
            # Split the input stream across two issue paths (HWDGE via
            # sync, SWDGE via gpsimd) so the SDMA engines interleave two
            # read queues with the scalar-ring write queue.
            eng = nc.gpsimd if (i % 2 == 1) else nc.sync
            eng.dma_start(xin[:], xt_r[:, :, t0:t0 + s])
            xins[i] = xin

        def stage_compute(i):
            if not (0 <= i < NSLAB):
                return
            t0, s = OFF[i], SLABS[i]
            ts = min(TILE, s)
            nt = s // ts
            xin = xins.pop(i)

            def xslice(j, c):
                return xin[:, c, j * ts:(j + 1) * ts]

            out_sb = out_sb_pool.tile([P, KB, s], FP8)
            pss = []
            for j in range(nt):
                for kb in range(KB):
                    ps = out_ps_pool.tile([P, ts], F32)
                    pss.append((j, kb, ps))
                    for c in range(C):
                        nc.tensor.matmul(
                            ps[:],
                            ct_sb[:, c, kb * P:(kb + 1) * P],
                            xslice(j, c),
                            start=(c == 0),
                            stop=(c == C - 1),
                        )
            for idx, (j, kb, ps) in enumerate(pss):
                dst = out_sb[:, kb, j * ts:(j + 1) * ts]
                if (idx + i) % 2 == 0:
                    nc.vector.tensor_copy(dst, ps[:])
                else:
                    nc.scalar.copy(dst, ps[:])
            if i >= NSLAB - 3:
                # Tail drain: per-tile, issued from the SYNC engine — its
                # input stream is done by now, so the dma issue cost
                # (~0.65us each) runs parallel to the scalar/DVE copies
                # instead of serializing behind them.
                for j in range(nt):
                    nc.sync.dma_start(
                        o_r[:, :, t0 + j * ts:t0 + (j + 1) * ts],
                        out_sb[:, :, j * ts:(j + 1) * ts],
                    )
            else:
                nc.scalar.dma_start(o_r[:, :, t0:t0 + s], out_sb[:])

        stage_in(0)
        stage_in(1)
        for i in range(NSLAB):
            stage_in(i + 2)
            stage_compute(i)

    nc.compile()
    return nc


_PROGRAM_CACHE: dict = {}


def _get_program() -> bass.Bass:
    if "nc" not in _PROGRAM_CACHE:
        _PROGRAM_CACHE["nc"] = build_program()
    return _PROGRAM_CACHE["nc"]


def make_in_maps(x_flat: np.ndarray) -> list[dict]:
    ct = np.ascontiguousarray(dct_matrix().T)  # [d, k] fp32
    ct_b = np.ascontiguousarray(
        ct.astype(BF16_NP).reshape(C, P, D).transpose(1, 0, 2)
    )  # [p, c, k]
    shards = x_flat.reshape(N_CORES, TOK, D)
    in_maps = []
    for i in range(N_CORES):
        xb = shards[i].astype(BF16_NP)                      # [TOK, D] bf16
        xt = np.ascontiguousarray(xb.T).reshape(C, P, TOK)  # [d, tok]
        in_maps.append({"xt": xt, "ct": ct_b})
    return in_maps


def kernel(x: np.ndarray) -> np.ndarray:
    x = np.ascontiguousarray(np.asarray(x, dtype=np.float32))
    b, n, d = x.shape
    assert (b, n, d) == (B, N, D), f"unexpected shape {x.shape}"
    nc = _get_program()
    in_maps = make_in_maps(x.reshape(b * n, d))
    res = run_bass_kernel_spmd(nc, in_maps, core_ids=list(range(N_CORES)))
    outs = []
    for r in res.results:
        o = np.asarray(r["out"]).reshape(D, TOK)   # [k, tok] fp8 e3m4
        outs.append(np.ascontiguousarray(o.T).astype(np.float32))
    return np.concatenate(outs, axis=0).reshape(b, n, d)


# revision 50
# speedup vs baseline: 1.0697x; 1.0697x over previous
"""DCT-II enhancement kernel for Trainium2 (8 NeuronCores, data parallel).

Computes out[b, n, k] = sum_d x[b, n, d] * C[k, d] where C is the 256x256
orthonormal DCT-II basis — i.e. a [B*N, 256] @ [256, 256]^T GEMM.

Sharding: pure data parallel over the flattened token dim (B*N = 131072),
16384 tokens per core.

Precision (harness gate: rel_err < 2e-2):
  x:   bf16  (input quantization ~1e-3)
  ct:  bf16  (basis must stay >= bf16; an fp8 basis hits e3m4 subnormals)
  acc: fp32 PSUM
  out: fp8 e3m4 (total rel err 1.363e-2, measured on the deterministic
       harness data — jax.random.key(0))
HBM traffic per core: 8.39 MB in + 4.19 MB out = 12.6 MB (vs 33.6 fp32).
(Sending a token fraction as fp8 with a split hi+lo e3m4 basis was tried
and is numerically fine at 1.52e-2, but the 2x PE streaming for those
tokens makes the kernel PE-bound and net ~8us slower — rejected.)

Layout trick: the host pre-transposes each shard to xT[d, tok] (d on
partitions) and post-transposes the result, so the device does NO
transposes — just matmuls with the tiny DCT basis stationary:

  outT[kb*128+kp, t] = sum_c ct_chunk[c, :, kb].T @ xT_chunk[c, :, t]

Per-core dataflow, per slab (graduated sizes 256..2048..512: small at
the head so the first matmul starts ~10us in, small at the tail to cut
the post-last-input drain chain):
  A(i): DMA xT tile [128p(d), 2c, S] from HBM (up to 4 KB runs per
        partition), alternating sync(HWDGE)/gpsimd(SWDGE) queues so the
        SDMA engines interleave two read streams with the write stream.
  B(i): per <=512-token tile x 2 k-blocks: 2 accumulating bf16 matmuls
        into a PSUM bank [128(k), <=512(t)]; PSUM fp32 -> SBUF fp8 cast
        copies (alternating DVE/ACT); out-DMA per slab on the scalar
        ring, per-tile on the (by then idle) sync ring for the last 3
        slabs.

Measured (core 0 NTFF, 8 cores concurrent): min 49.0us, median ~53us
over 8 samples, vs 104.5us for the fp32 baseline. Steady-state DMA is
99%+ packed at ~341 GB/s; the remaining fixed costs are ~3us of ramp
and ~8.5us of tail (final DMA receipt + the codegen-emitted
253-semaphore clear postamble).
"""

from contextlib import ExitStack

import ml_dtypes
import numpy as np

import concourse.bass as bass
import concourse.tile as tile
from concourse import bacc, mybir
from concourse.bass_utils import run_bass_kernel_spmd

P = 128
D = 256
N_CORES = 8
B, N = 32, 4096
TOK = (B * N) // N_CORES  # 16384 tokens per core
C = D // P                # 2 contraction chunks of 128
KB = D // P               # 2 output k-blocks of 128

TILE = 512                # tokens per PSUM tile (one bank: 512 fp32)
# Graduated slabs: small at the head (fast pipeline fill -> first matmul
# sooner) and at the tail (short post-last-input drain chain).
SLABS = [256, 256, 512, 1024] + [2048] * 6 + [1024, 512, 512]  # sum 16384
OFF = [sum(SLABS[:i]) for i in range(len(SLABS))]
NSLAB = len(SLABS)

BF16 = mybir.dt.bfloat16
F32 = mybir.dt.float32
FP8 = mybir.dt.float8e3

BF16_NP = ml_dtypes.bfloat16
FP8_NP = ml_dtypes.float8_e3m4  # wire format of the "out" tensor


def dct_matrix() -> np.ndarray:
    """C[k, d] — DCT-II with ortho normalization, fp64 math cast to fp32."""
    n = D
    k = np.arange(n)[:, None].astype(np.float64)
    m = np.arange(n)[None, :].astype(np.float64)
    Cm = np.cos(np.pi * (2.0 * m + 1.0) * k / (2.0 * n))
    scale = np.full((n, 1), np.sqrt(2.0 / n))
    scale[0, 0] = np.sqrt(1.0 / n)
    return (Cm * scale).astype(np.float32)


def build_program(num_devices: int = N_CORES) -> bass.Bass:
    """Emit the per-core Bass/Tile program. All cores run the same NEFF."""
    nc = bacc.Bacc(
        "TRN2", target_bir_lowering=False, debug=False, num_devices=num_devices
    )
    xt_d = nc.dram_tensor("xt", [C, P, TOK], BF16, kind="ExternalInput").ap()
    # ct packed [p, c, k] host-side: one contiguous 1 KB run per partition.
    ct_d = nc.dram_tensor("ct", [P, C, D], BF16, kind="ExternalInput").ap()
    out_d = nc.dram_tensor("out", [KB, P, TOK], FP8, kind="ExternalOutput").ap()

    with ExitStack() as ctx:
        tc = ctx.enter_context(tile.TileContext(nc))
        consts = ctx.enter_context(tc.tile_pool(name="consts", bufs=1))
        xin_pool = ctx.enter_context(tc.tile_pool(name="xin", bufs=4))
        out_sb_pool = ctx.enter_context(tc.tile_pool(name="out_sb", bufs=4))
        out_ps_pool = ctx.enter_context(
            tc.tile_pool(name="out_ps", bufs=8, space="PSUM")
        )

        # Basis on the scalar ring so the sync ring starts streaming x
        # immediately; flat layout -> fast descriptors -> first MM sooner.
        ct_sb = consts.tile([P, C, D], BF16)
        nc.scalar.dma_start(ct_sb[:], ct_d)

        xt_r = xt_d.rearrange("c p t -> p c t")    # [128, 2, TOK]
        o_r = out_d.rearrange("c p t -> p c t")    # [128, 2, TOK]

        xins: dict = {}

        def stage_in(i):
            if not (0 <= i < NSLAB):
                return
            t0, s = OFF[i], SLABS[i]
            xin = xin_pool.tile([P, C, s], BF16)
            # Split the input stream across two issue paths (HWDGE via
            # sync, SWDGE via gpsimd) so the SDMA engines interleave two
            # read queues with the scalar-ring write queue.
            eng = nc.gpsimd if (i % 2 == 1) else nc.sync
            eng.dma_start(xin[:], xt_r[:, :, t0:t0 + s])
            xins[i] = xin

        def stage_compute(i):
            if not (0 <= i < NSLAB):
                return
            t0, s = OFF[i], SLABS[i]
            ts = min(TILE, s)
            nt = s // ts
            xin = xins.pop(i)

            def xslice(j, c):
                return xin[:, c, j * ts:(j + 1) * ts]

            out_sb = out_sb_pool.tile([P, KB, s], FP8)
            pss = []
            for j in range(nt):
                for kb in range(KB):
                    ps = out_ps_pool.tile([P, ts], F32)
                    pss.append((j, kb, ps))
                    for c in range(C):
                        nc.tensor.matmul(
                            ps[:],
                            ct_sb[:, c, kb * P:(kb + 1) * P],
                            xslice(j, c),
                            start=(c == 0),
                            stop=(c == C - 1),
                        )
            for idx, (j, kb, ps) in enumerate(pss):
                dst = out_sb[:, kb, j * ts:(j + 1) * ts]
                if (idx + i) % 2 == 0:
                    nc.vector.tensor_copy(dst, ps[:])
                else:
                    nc.scalar.copy(dst, ps[:])
            if i >= NSLAB - 3:
                # Tail drain: per-tile, issued from the SYNC engine — its
                # input stream is done by now, so the dma issue cost
                # (~0.65us each) runs parallel to the scalar/DVE copies
                # instead of serializing behind them.
                for j in range(nt):
                    nc.sync.dma_start(
                        o_r[:, :, t0 + j * ts:t0 + (j + 1) * ts],
                        out_sb[:, :, j * ts:(j + 1) * ts],
                    )
            else:
                # Issue mid-run output DMAs from gpsimd (SWDGE, otherwise
                # ~85% idle) so the scalar engine's FIFO carries only ACT
                # copies — its dma issues (~0.65us each) otherwise queue
                # ahead of copies and delay the drain chain.
                nc.gpsimd.dma_start(o_r[:, :, t0:t0 + s], out_sb[:])

        stage_in(0)
        stage_in(1)
        for i in range(NSLAB):
            stage_in(i + 2)
            stage_compute(i)

    nc.compile()
    return nc


_PROGRAM_CACHE: dict = {}


def _get_program() -> bass.Bass:
    if "nc" not in _PROGRAM_CACHE:
        _PROGRAM_CACHE["nc"] = build_program()
    return _PROGRAM_CACHE["nc"]


def make_in_maps(x_flat: np.ndarray) -> list[dict]:
    ct = np.ascontiguousarray(dct_matrix().T)  # [d, k] fp32
    ct_b = np.ascontiguousarray(
        ct.astype(BF16_NP).reshape(C, P, D).transpose(1, 0, 2)
    )  # [p, c, k]
    shards = x_flat.reshape(N_CORES, TOK, D)
    in_maps = []
    for i in range(N_CORES):
        xb = shards[i].astype(BF16_NP)                      # [TOK, D] bf16
        xt = np.ascontiguousarray(xb.T).reshape(C, P, TOK)  # [d, tok]
        in_maps.append({"xt": xt, "ct": ct_b})
    return in_maps


def kernel(x: np.ndarray) -> np.ndarray:
    x = np.ascontiguousarray(np.asarray(x, dtype=np.float32))
    b, n, d = x.shape
    assert (b, n, d) == (B, N, D), f"unexpected shape {x.shape}"
    nc = _get_program()
    in_maps = make_in_maps(x.reshape(b * n, d))
    res = run_bass_kernel_spmd(nc, in_maps, core_ids=list(range(N_CORES)))
    outs = []
    for r in res.results:
        o = np.asarray(r["out"]).reshape(D, TOK)   # [k, tok] fp8 e3m4
        outs.append(np.ascontiguousarray(o.T).astype(np.float32))
    return np.concatenate(outs, axis=0).reshape(b, n, d)


# revision 51
# speedup vs baseline: 1.0764x; 1.0063x over previous
"""DCT-II enhancement kernel for Trainium2 (8 NeuronCores, data parallel).

Computes out[b, n, k] = sum_d x[b, n, d] * C[k, d] where C is the 256x256
orthonormal DCT-II basis — i.e. a [B*N, 256] @ [256, 256]^T GEMM.

Sharding: pure data parallel over the flattened token dim (B*N = 131072),
16384 tokens per core.

Precision (harness gate: rel_err < 2e-2):
  x:   bf16  (input quantization ~1e-3)
  ct:  bf16  (basis must stay >= bf16; an fp8 basis hits e3m4 subnormals)
  acc: fp32 PSUM
  out: fp8 e3m4 (total rel err 1.363e-2, measured on the deterministic
       harness data — jax.random.key(0))
HBM traffic per core: 8.39 MB in + 4.19 MB out = 12.6 MB (vs 33.6 fp32).
(Sending a token fraction as fp8 with a split hi+lo e3m4 basis was tried
and is numerically fine at 1.52e-2, but the 2x PE streaming for those
tokens makes the kernel PE-bound and net ~8us slower — rejected.)

Layout trick: the host pre-transposes each shard to xT[d, tok] (d on
partitions) and post-transposes the result, so the device does NO
transposes — just matmuls with the tiny DCT basis stationary:

  outT[kb*128+kp, t] = sum_c ct_chunk[c, :, kb].T @ xT_chunk[c, :, t]

Per-core dataflow, per slab (graduated sizes 256..2048..512: small at
the head so the first matmul starts ~10us in, small at the tail to cut
the post-last-input drain chain):
  A(i): DMA xT tile [128p(d), 2c, S] from HBM (up to 4 KB runs per
        partition), alternating sync(HWDGE)/gpsimd(SWDGE) queues so the
        SDMA engines interleave two read streams with the write stream.
  B(i): per <=512-token tile x 2 k-blocks: 2 accumulating bf16 matmuls
        into a PSUM bank [128(k), <=512(t)]; PSUM fp32 -> SBUF fp8 cast
        copies (alternating DVE/ACT); out-DMA per slab on the scalar
        ring, per-tile on the (by then idle) sync ring for the last 3
        slabs.

Measured (core 0 NTFF, 8 cores concurrent): min 49.0us, median ~53us
over 8 samples, vs 104.5us for the fp32 baseline. Steady-state DMA is
99%+ packed at ~341 GB/s; the remaining fixed costs are ~3us of ramp
and ~8.5us of tail (final DMA receipt + the codegen-emitted
253-semaphore clear postamble).
"""

from contextlib import ExitStack

import ml_dtypes
import numpy as np

import concourse.bass as bass
import concourse.tile as tile
from concourse import bacc, mybir
from concourse.bass_utils import run_bass_kernel_spmd

P = 128
D = 256
N_CORES = 8
B, N = 32, 4096
TOK = (B * N) // N_CORES  # 16384 tokens per core
C = D // P                # 2 contraction chunks of 128
KB = D // P               # 2 output k-blocks of 128

TILE = 512                # tokens per PSUM tile (one bank: 512 fp32)
# Graduated slabs: small at the head (fast pipeline fill -> first matmul
# sooner) and at the tail (short post-last-input drain chain).
SLABS = [256, 256, 512, 1024] + [2048] * 6 + [1024, 512, 512]  # sum 16384
OFF = [sum(SLABS[:i]) for i in range(len(SLABS))]
NSLAB = len(SLABS)

BF16 = mybir.dt.bfloat16
F32 = mybir.dt.float32
FP8 = mybir.dt.float8e3

BF16_NP = ml_dtypes.bfloat16
FP8_NP = ml_dtypes.float8_e3m4  # wire format of the "out" tensor


def dct_matrix() -> np.ndarray:
    """C[k, d] — DCT-II with ortho normalization, fp64 math cast to fp32."""
    n = D
    k = np.arange(n)[:, None].astype(np.float64)
    m = np.arange(n)[None, :].astype(np.float64)
    Cm = np.cos(np.pi * (2.0 * m + 1.0) * k / (2.0 * n))
    scale = np.full((n, 1), np.sqrt(2.0 / n))
    scale[0, 0] = np.sqrt(1.0 / n)
    return (Cm * scale).astype(np.float32)


def build_program(num_devices: int = N_CORES) -> bass.Bass:
    """Emit the per-core Bass/Tile program. All cores run the same NEFF."""
    nc = bacc.Bacc(
        "TRN2", target_bir_lowering=False, debug=False, num_devices=num_devices
    )
    xt_d = nc.dram_tensor("xt", [C, P, TOK], BF16, kind="ExternalInput").ap()
    # ct packed [p, c, k] host-side: one contiguous 1 KB run per partition.
    ct_d = nc.dram_tensor("ct", [P, C, D], BF16, kind="ExternalInput").ap()
    out_d = nc.dram_tensor("out", [KB, P, TOK], FP8, kind="ExternalOutput").ap()

    with ExitStack() as ctx:
        tc = ctx.enter_context(tile.TileContext(nc))
        consts = ctx.enter_context(tc.tile_pool(name="consts", bufs=1))
        xin_pool = ctx.enter_context(tc.tile_pool(name="xin", bufs=4))
        out_sb_pool = ctx.enter_context(tc.tile_pool(name="out_sb", bufs=4))
        out_ps_pool = ctx.enter_context(
            tc.tile_pool(name="out_ps", bufs=8, space="PSUM")
        )

        # Basis on the scalar ring so the sync ring starts streaming x
        # immediately; flat layout -> fast descriptors -> first MM sooner.
        ct_sb = consts.tile([P, C, D], BF16)
        nc.scalar.dma_start(ct_sb[:], ct_d)

        xt_r = xt_d.rearrange("c p t -> p c t")    # [128, 2, TOK]
        o_r = out_d.rearrange("c p t -> p c t")    # [128, 2, TOK]

        xins: dict = {}

        def stage_in(i):
            if not (0 <= i < NSLAB):
                return
            t0, s = OFF[i], SLABS[i]
            xin = xin_pool.tile([P, C, s], BF16)
            # Split the input stream across two issue paths (HWDGE via
            # sync, SWDGE via gpsimd) so the SDMA engines interleave two
            # read queues with the scalar-ring write queue.
            eng = nc.gpsimd if (i % 2 == 1) else nc.sync
            eng.dma_start(xin[:], xt_r[:, :, t0:t0 + s])
            xins[i] = xin

        def stage_compute(i):
            if not (0 <= i < NSLAB):
                return
            t0, s = OFF[i], SLABS[i]
            ts = min(TILE, s)
            nt = s // ts
            xin = xins.pop(i)

            def xslice(j, c):
                return xin[:, c, j * ts:(j + 1) * ts]

            out_sb = out_sb_pool.tile([P, KB, s], FP8)
            pss = []
            for j in range(nt):
                for kb in range(KB):
                    ps = out_ps_pool.tile([P, ts], F32)
                    pss.append((j, kb, ps))
                    for c in range(C):
                        nc.tensor.matmul(
                            ps[:],
                            ct_sb[:, c, kb * P:(kb + 1) * P],
                            xslice(j, c),
                            start=(c == 0),
                            stop=(c == C - 1),
                        )
            for idx, (j, kb, ps) in enumerate(pss):
                dst = out_sb[:, kb, j * ts:(j + 1) * ts]
                if (idx + i) % 2 == 0:
                    nc.vector.tensor_copy(dst, ps[:])
                else:
                    nc.scalar.copy(dst, ps[:])
            if i >= NSLAB - 3:
                # Tail drain: per-tile, issued from the SYNC engine — its
                # input stream is done by now, so the dma issue cost
                # (~0.65us each) runs parallel to the scalar/DVE copies
                # instead of serializing behind them.
                for j in range(nt):
                    nc.sync.dma_start(
                        o_r[:, :, t0 + j * ts:t0 + (j + 1) * ts],
                        out_sb[:, :, j * ts:(j + 1) * ts],
                    )
            else:
                nc.scalar.dma_start(o_r[:, :, t0:t0 + s], out_sb[:])

        stage_in(0)
        stage_in(1)
        for i in range(NSLAB):
            stage_in(i + 2)
            stage_compute(i)

    nc.compile()
    return nc


_PROGRAM_CACHE: dict = {}


def _get_program() -> bass.Bass:
    if "nc" not in _PROGRAM_CACHE:
        _PROGRAM_CACHE["nc"] = build_program()
    return _PROGRAM_CACHE["nc"]


def make_in_maps(x_flat: np.ndarray) -> list[dict]:
    ct = np.ascontiguousarray(dct_matrix().T)  # [d, k] fp32
    ct_b = np.ascontiguousarray(
        ct.astype(BF16_NP).reshape(C, P, D).transpose(1, 0, 2)
    )  # [p, c, k]
    shards = x_flat.reshape(N_CORES, TOK, D)
    in_maps = []
    for i in range(N_CORES):
        xb = shards[i].astype(BF16_NP)                      # [TOK, D] bf16
        xt = np.ascontiguousarray(xb.T).reshape(C, P, TOK)  # [d, tok]
        in_maps.append({"xt": xt, "ct": ct_b})
    return in_maps


def kernel(x: np.ndarray) -> np.ndarray:
    x = np.ascontiguousarray(np.asarray(x, dtype=np.float32))
    b, n, d = x.shape
    assert (b, n, d) == (B, N, D), f"unexpected shape {x.shape}"
    nc = _get_program()
    in_maps = make_in_maps(x.reshape(b * n, d))
    res = run_bass_kernel_spmd(nc, in_maps, core_ids=list(range(N_CORES)))
    outs = []
    for r in res.results:
        o = np.asarray(r["out"]).reshape(D, TOK)   # [k, tok] fp8 e3m4
        outs.append(np.ascontiguousarray(o.T).astype(np.float32))
    return np.concatenate(outs, axis=0).reshape(b, n, d)
